# revision 29
# baseline (speedup 1.0000x reference)
"""BitLinearLRLS fused kernel for 8 Trainium2 NeuronCores.

Math (see reference):
    w_q       = clip(round(weight / 0.5), -1, 1)            # ternary, RNE ties
    x_mean    = mean(x, axis=(0,1))                         # [in]
    scale_eff = scale + lrls_A @ (lrls_B @ x_mean)          # [out]
    y         = x @ (w_q * scale_eff[:, None]).T

Key identity: y = (x @ w_q.T) * scale_eff[None, :] — the big matmul does not
depend on scale_eff, so the data-dependent scale is a per-output-row epilogue
on PSUM tiles (ACT Copy with per-partition scale).

Sharding: data-parallel over tokens. Each of the 8 cores takes tok/8 tokens
(x transposed on host to [in, tokens] so the contraction dim lands on SBUF
partitions), keeps a quantized fp32r weight slab resident, and computes its
y^T slice. Per-core token sums are AllReduce'd (16 KiB) for x_mean.

Engine plan (engine queues are FIFO — a blocked op stalls everything behind
it on that engine, so each stream owns one engine/ring):
  PE      : the 4096 [128x128x512] fp32r matmuls, nothing else
  DVE     : weight quantize (mult+max / min+int32cast / cast-to-f32r), then
            the tiny LRLS matvec chain (pure DVE via broadcast DMAs)
  ACT     : pass-1 token sums (self-copy with accum_out), PSUM epilogues
  sync SP : x-tile loads for the matmuls (f32r DMA-bitcast, no convert)
  gpsimd  : weight slab loads (SWDGE), half the pass-1 loads, collective
  scalar  : the other half of pass-1 loads, y stores, small DMAs

Quantization is exact vs the reference:
    w_q = int32_cast_rne(clamp(2w, -1.25, 1.25))
(the DVE float->int cast is round-to-nearest-even; verified on HW, including
ties: round(+-0.5) -> 0). is_gt/is_lt ALU ops are avoided — they run ~20x
slower than mult/max/min on the DVE.

The matmul runs in float32r (TF32-like, 1 cycle/row at N=512): weights in
{-1,0,1} are exact in fp32r; only x carries the ~2^-12 read rounding. x is
fed to the PE by declaring its DRAM tensor float32r and DMA-ing straight into
float32r tiles (bit layout is fp32-compatible, the PE rounds on read —
verified on HW to match a DVE-converted operand), so no per-tile cast pass.
"""

import numpy as np

import concourse.bass as bass
import concourse.tile as tile
from concourse import bacc, mybir
from concourse.bass_utils import run_bass_kernel_spmd

F32 = mybir.dt.float32
F32R = mybir.dt.float32r
I32 = mybir.dt.int32
ALU = mybir.AluOpType
ACTF = mybir.ActivationFunctionType


class Cfg:
    def __init__(self, tok=16384, din=4096, dout=4096, r=16,
                 tsh=None, oslab=1024, tblk=512, ncores=8):
        self.ncores = ncores
        self.tok = tok            # total tokens (B*S)
        self.din = din
        self.dout = dout
        self.r = r
        self.tsh = tsh or tok // ncores   # tokens per core
        self.oslab = oslab        # output features per resident W slab
        self.tblk = tblk          # moving-operand tile (tokens)
        self.kc = din // 128      # contraction chunks
        self.oc = dout // 128     # output chunks (scale_eff columns)
        self.nslab = dout // self.oslab
        self.ntblk = self.tsh // tblk
        self.nos = self.oslab // 128  # psum banks per t-block
        self.bchunk = min(512, din // 4)   # LRLS B matvec chunk
        assert self.nos <= 8 and din % self.bchunk == 0
        # first slab sized so all its (t-block, o-chunk) psum groups fit in
        # the 8 banks simultaneously — no bank recycling, so its epilogues
        # (which need the AllReduce'd scale) gate nothing
        self.osize0 = min((8 // self.ntblk) * 128, self.oslab)
        slabs = [(0, self.osize0)]
        if self.oslab - self.osize0:
            slabs.append((self.osize0, self.oslab - self.osize0))
        slabs += [(s * self.oslab, self.oslab) for s in range(1, self.nslab)]
        self.slabs = slabs


def build(cfg: Cfg, compile=True):
    nc = bacc.Bacc("TRN2", target_bir_lowering=False, debug=False,
                   enable_asserts=True, num_devices=cfg.ncores)

    xT = nc.dram_tensor("xT", [cfg.din, cfg.tsh], F32R,
                        kind="ExternalInput").ap()
    wT = nc.dram_tensor("wT", [cfg.din, cfg.dout], F32,
                        kind="ExternalInput").ap()
    scale_pc = nc.dram_tensor("scale_pc", [128, cfg.oc], F32,
                              kind="ExternalInput").ap()
    # b_pk[r, p*kc + k] = B[r, k*128 + p]  (matches sums' [p, k] flat order)
    b_pk = nc.dram_tensor("b_pk", [cfg.r, cfg.din], F32,
                          kind="ExternalInput").ap()
    # a_p[p, j*r + rr] = A[j*128 + p, rr]
    a_p = nc.dram_tensor("a_p", [128, cfg.oc * cfg.r], F32,
                         kind="ExternalInput").ap()
    yT = nc.dram_tensor("yT", [cfg.dout, cfg.tsh], F32,
                        kind="ExternalOutput").ap()

    with tile.TileContext(nc) as tc:
        with tc.tile_pool(name="keep", bufs=1) as keep, \
             tc.tile_pool(name="cdram", bufs=1, space="DRAM") as cdram, \
             tc.tile_pool(name="wq", bufs=1) as wqp, \
             tc.tile_pool(name="wst", bufs=3) as wst, \
             tc.tile_pool(name="qa", bufs=1) as qap, \
             tc.tile_pool(name="qb", bufs=2) as qbp, \
             tc.tile_pool(name="xst", bufs=8) as xst, \
             tc.tile_pool(name="yst", bufs=1) as yst, \
             tc.tile_pool(name="lr", bufs=1) as lr, \
             tc.tile_pool(name="lr2", bufs=2) as lr2:

            sums = keep.tile([128, cfg.kc], F32)
            scale_eff = keep.tile([128, cfg.oc], F32)
            partials = keep.tile([128, cfg.kc, cfg.ntblk], F32)

            def quantize_slab(ostart, osize):
                wq_sb = wqp.tile([128, cfg.kc, osize], F32R, name="wq_sb")
                for ib in range(cfg.kc):
                    wr = wst.tile([128, osize], F32, name="wr")
                    nc.gpsimd.dma_start(
                        out=wr,
                        in_=wT[ib * 128:(ib + 1) * 128,
                               ostart:ostart + osize])
                    ga = qap.tile([128, osize], F32, name="ga")
                    nc.vector.tensor_scalar(
                        out=ga, in0=wr, scalar1=2.0, scalar2=-1.25,
                        op0=ALU.mult, op1=ALU.max)
                    gb = qbp.tile([128, osize], I32, name="gb")
                    nc.vector.tensor_scalar(
                        out=gb, in0=ga, scalar1=1.25, scalar2=None,
                        op0=ALU.min)
                    nc.vector.tensor_copy(out=wq_sb[:, ib, :], in_=gb)
                return wq_sb

            def epilogue(ostart, osize, tb, pst):
                nos = osize // 128
                yt = yst.tile([128, nos, cfg.tblk], F32, name="yt")
                for os_ in range(nos):
                    oi = ostart // 128 + os_
                    nc.scalar.activation(
                        out=yt[:, os_, :], in_=pst[os_],
                        func=ACTF.Copy,
                        scale=scale_eff[:, oi:oi + 1],
                    )
                nc.scalar.dma_start(
                    out=yT[ostart:ostart + osize,
                           tb * cfg.tblk:(tb + 1) * cfg.tblk]
                    .rearrange("(c p) t -> p c t", p=128),
                    in_=yt)

            def tb_loop(psp, ostart, osize, wq_sb, with_reduce=False,
                        defer_epilogue=False, split_rings=False):
                nos = osize // 128
                deferred = []
                for tb in range(cfg.ntblk):
                    pst = [psp.tile([128, cfg.tblk], F32, name="pst")
                           for _ in range(nos)]
                    for i in range(cfg.kc):
                        xr = xst.tile([128, cfg.tblk], F32R, name="xr")
                        # the sync HWDGE ring tops out ~200 GB/s (descriptor
                        # rate); during slab 0a gpsimd is idle, so split
                        eng = nc.gpsimd if (split_rings and i % 2) else nc.sync
                        eng.dma_start(
                            out=xr,
                            in_=xT[i * 128:(i + 1) * 128,
                                   tb * cfg.tblk:(tb + 1) * cfg.tblk])
                        if with_reduce:
                            nc.vector.reduce_sum(
                                out=partials[:, i, tb:tb + 1],
                                in_=xr.bitcast(F32),
                                axis=mybir.AxisListType.X)
                        for os_ in range(nos):
                            nc.tensor.matmul(
                                pst[os_],
                                lhsT=wq_sb[:, i, os_ * 128:(os_ + 1) * 128],
                                rhs=xr,
                                start=(i == 0), stop=(i == cfg.kc - 1),
                            )
                    if defer_epilogue:
                        deferred.append((tb, pst))
                    else:
                        epilogue(ostart, osize, tb, pst)
                return deferred

            with tc.tile_pool(name="ps", bufs=8, space="PSUM") as psp:
                # ---- slab 0a: matmuls + fused token-sum reduces. Its x
                # stream doubles as the x_mean pass; all psum groups fit the
                # 8 banks, so its (scale-gated) epilogues block nothing. ----
                ostart0, osize0 = cfg.slabs[0]
                wq_sb = quantize_slab(ostart0, osize0)
                deferred = tb_loop(psp, ostart0, osize0, wq_sb,
                                   with_reduce=True, defer_epilogue=True,
                                   split_rings=True)
                nc.vector.reduce_sum(out=sums, in_=partials,
                                     axis=mybir.AxisListType.X)

                # ---- AllReduce the partial sums across the 8 cores.
                # cc_in rides the sync ring (ACT would deadlock behind the
                # scale-gated epilogues; gpsimd would starve W loads). ----
                cc_in = cdram.tile([128, cfg.kc], F32)
                cc_out = cdram.tile([128, cfg.kc], F32)
                v_d = cdram.tile([1, cfg.r], F32)
                nc.sync.dma_start(out=cc_in, in_=sums)
                nc.gpsimd.collective_compute(
                    "AllReduce", ALU.add,
                    replica_groups=[list(range(cfg.ncores))],
                    ins=[cc_in.opt()], outs=[cc_out.opt()],
                )

                # ---- scale_eff = scale + A @ (B @ sum_x) / tok, pure DVE,
                # no PSUM (the 8 banks belong to the matmul pipeline) ----
                nchunk = cfg.din // cfg.bchunk
                vparts = lr.tile([cfg.r, nchunk], F32)
                for c in range(nchunk):
                    xbc = lr2.tile([cfg.r, cfg.bchunk], F32, name="xbc")
                    nc.gpsimd.dma_start(
                        out=xbc,
                        in_=bass.AP(tensor=cc_out.tensor,
                                    offset=cc_out.offset + c * cfg.bchunk,
                                    ap=[[0, cfg.r], [1, cfg.bchunk]]))
                    bsb = lr2.tile([cfg.r, cfg.bchunk], F32, name="bsb")
                    nc.gpsimd.dma_start(
                        out=bsb,
                        in_=b_pk[:, c * cfg.bchunk:(c + 1) * cfg.bchunk])
                    nc.vector.tensor_tensor(out=xbc, in0=bsb, in1=xbc,
                                            op=ALU.mult)
                    nc.vector.reduce_sum(out=vparts[:, c:c + 1], in_=xbc,
                                         axis=mybir.AxisListType.X)
                vsb = lr.tile([cfg.r, 1], F32)
                nc.vector.reduce_sum(out=vsb, in_=vparts,
                                     axis=mybir.AxisListType.X)
                nc.gpsimd.dma_start(out=v_d.rearrange("one r -> r one"),
                                  in_=vsb)

                # corr[p, j] = sum_rr a_p[p, j*r+rr] * v[rr]
                vb16 = lr.tile([128, cfg.r], F32)
                nc.gpsimd.dma_start(
                    out=vb16,
                    in_=bass.AP(tensor=v_d.tensor, offset=v_d.offset,
                                ap=[[0, 128], [1, cfg.r]]))
                ap_sb = lr.tile([128, cfg.oc, cfg.r], F32)
                nc.gpsimd.dma_start(
                    out=ap_sb,
                    in_=a_p.rearrange("p (j rr) -> p j rr", rr=cfg.r))
                am = lr.tile([128, cfg.oc, cfg.r], F32)
                for j in range(cfg.oc):
                    nc.vector.tensor_tensor(out=am[:, j, :],
                                            in0=ap_sb[:, j, :],
                                            in1=vb16, op=ALU.mult)
                corr = lr.tile([128, cfg.oc], F32)
                nc.vector.reduce_sum(out=corr, in_=am,
                                     axis=mybir.AxisListType.X)
                sc_sb = lr.tile([128, cfg.oc], F32)
                nc.gpsimd.dma_start(out=sc_sb, in_=scale_pc)
                nc.vector.tensor_scalar(out=scale_eff, in0=corr,
                                        scalar1=1.0 / cfg.tok, scalar2=None,
                                        op0=ALU.mult)
                nc.vector.tensor_tensor(out=scale_eff, in0=scale_eff,
                                        in1=sc_sb, op=ALU.add)

                # slab 0a's deferred epilogues, now that scale_eff exists
                for tb, pst in deferred:
                    epilogue(ostart0, osize0, tb, pst)

                # ---- remaining slabs ----
                for ostart, osize in cfg.slabs[1:]:
                    wq_sb = quantize_slab(ostart, osize)
                    tb_loop(psp, ostart, osize, wq_sb)

    if compile:
        nc.compile()
    return nc


def prep_inputs(cfg: Cfg, x, weight, scale, lrls_A, lrls_B):
    """Host-side sharding/layout marshalling (no arithmetic on the data)."""
    x_flat = np.ascontiguousarray(x.reshape(cfg.tok, cfg.din))
    xT_full = np.ascontiguousarray(x_flat.T)          # [din, tok]
    wT = np.ascontiguousarray(weight.T)               # [din, dout]
    b_pk = np.ascontiguousarray(
        lrls_B.reshape(cfg.r, cfg.kc, 128).transpose(0, 2, 1).reshape(
            cfg.r, cfg.din))
    a_p = np.ascontiguousarray(
        lrls_A.reshape(cfg.oc, 128, cfg.r).transpose(1, 0, 2).reshape(
            128, cfg.oc * cfg.r))
    scale_pc = np.ascontiguousarray(scale.reshape(cfg.oc, 128).T)

    in_maps = []
    for c in range(cfg.ncores):
        xT_c = np.ascontiguousarray(
            xT_full[:, c * cfg.tsh:(c + 1) * cfg.tsh])
        in_maps.append({"xT": xT_c, "wT": wT, "scale_pc": scale_pc,
                        "b_pk": b_pk, "a_p": a_p})
    return in_maps


def assemble_output(cfg: Cfg, results, out_shape):
    y_flat = np.empty((cfg.tok, cfg.dout), np.float32)
    for c in range(cfg.ncores):
        y_flat[c * cfg.tsh:(c + 1) * cfg.tsh, :] = results[c]["yT"].T
    return y_flat.reshape(out_shape)


_NC_CACHE = {}


def run(cfg: Cfg, x, weight, scale, lrls_A, lrls_B, out_shape, **run_kwargs):
    key = (cfg.tok, cfg.din, cfg.dout, cfg.tsh, cfg.oslab, cfg.tblk)
    if key not in _NC_CACHE:
        _NC_CACHE[key] = build(cfg)
    nc = _NC_CACHE[key]
    in_maps = prep_inputs(cfg, x, weight, scale, lrls_A, lrls_B)
    res = run_bass_kernel_spmd(nc, in_maps, core_ids=list(range(cfg.ncores)),
                               **run_kwargs)
    y = assemble_output(cfg, res.results, out_shape)
    return y, res


def kernel(x, weight, threshold, scale, lrls_A, lrls_B):
    # threshold input is unused: the reference hardcodes THRESH=0.5
    # (TrainState.threshold() at step 0), so the ternary cut sits at |w|=0.25.
    cfg = Cfg()
    x = np.asarray(x, np.float32)
    y, _ = run(cfg, x, np.asarray(weight, np.float32),
               np.asarray(scale, np.float32), np.asarray(lrls_A, np.float32),
               np.asarray(lrls_B, np.float32),
               out_shape=(x.shape[0], x.shape[1], np.asarray(weight).shape[0]))
    return y.astype(np.float32)
